# revision 3
# baseline (speedup 1.0000x reference)
"""Decoder-only attention block (QKV proj + MHA + out proj) on 8 TRN2 cores.

Sharding: core c -> (batch b = c//4, head-group g = c%4). Tensor-parallel over
heads (4 of 16 heads per core), data-parallel over batch (2). Each core
computes a partial c_proj over its 512 input features; host reduces the 4
partials per batch and adds biases.

Self-contained: hardcodes B=2, S=2048, D=2048, H=16.
"""

import os

import numpy as np

NPF16 = np.float16

import concourse.bass as bass
import concourse.bacc as bacc
import concourse.tile as tile
from concourse import mybir
import concourse.bass_utils as bass_utils
import concourse.bass_isa as bass_isa
from concourse.bass_interp import get_hw_module

B, S, D = 2, 2048, 2048
H, DH = 16, 128
N_CORES = 8
HL = H // 4            # 4 heads per core
FL = HL * DH           # 512 local features per core
KT = D // 128          # 16 contraction tiles
TT = S // 128          # 16 token tiles
QB = S // 512          # 4 token blocks
SCALE = 1.0 / float(np.sqrt(DH))

F16 = mybir.dt.float16
F32 = mybir.dt.float32

# Stash of the last BassKernelResults (for the local test harness only).
LAST_RESULTS = None
_PROG_CACHE = {}


def _build_program(use_mask):

    nc = bacc.Bacc("TRN2", target_bir_lowering=False, debug=False,
                   num_devices=N_CORES)

    # kt-major x for QK (moving operand), t-major copy for V (stationary).
    xt_d = nc.dram_tensor("xt", [128, KT * S], F16, kind="ExternalInput")
    xv_d = nc.dram_tensor("xv", [128, TT * S], F16, kind="ExternalInput")
    # head-pair-major QK weights: [row, h*4096 + kt*256 + half*128 + col]
    wqk_d = nc.dram_tensor("wqk", [128, HL * 4096], F16, kind="ExternalInput")
    wv_d = nc.dram_tensor("wv", [128, KT * FL], F16, kind="ExternalInput")
    wp_d = nc.dram_tensor("wp", [128, HL * D], F16, kind="ExternalInput")
    bqk_d = nc.dram_tensor("bqk", [128, 8], F32, kind="ExternalInput")
    kb_d = nc.dram_tensor("kb", [128, KT], F32, kind="ExternalInput")
    out_d = nc.dram_tensor("out", [S, D], F16, kind="ExternalOutput")

    xt_ap, xv_ap, wqk_ap = xt_d.ap(), xv_d.ap(), wqk_d.ap()
    wv_ap, wp_ap = wv_d.ap(), wp_d.ap()
    bqk_ap, kb_ap, out_ap = bqk_d.ap(), kb_d.ap(), out_d.ap()

    with tile.TileContext(nc) as tc, tc.tile_pool(name="pers", bufs=1) as pers:
        # ---- persistent tiles (live across phases) ----
        qt = [pers.tile([128, S], F16, tag=f"qt{h}", name=f"qt{h}") for h in range(HL)]
        ktt = [pers.tile([128, S], F16, tag=f"kt{h}", name=f"ktt{h}") for h in range(HL)]
        ot = [pers.tile([128, S], F16, tag=f"ot{h}", name=f"ot{h}") for h in range(HL)]
        vaug = [[pers.tile([128, DH], F16, tag=f"v{t}_{h}", name=f"v{t}_{h}")
                 for h in range(HL)] for t in range(TT)]
        wp_sb = pers.tile([128, HL * D], F16, tag="wp", name="wp_sb")
        # wv + rolling xv buffers live in the persistent pool so their DMA
        # writes don't alias phase-1 space (aliasing would chain them behind
        # the last QK matmul).
        wv_sb = pers.tile([128, KT * FL], F16, tag="wv", name="wv_sb")
        bqk_sb = pers.tile([128, 8], F32, tag="bqk", name="bqk_sb")
        kb_sb = pers.tile([128, KT], F32, tag="kb", name="kb_sb")
        warm = pers.tile([128, 512], F16, tag="warm", name="warm")

        # small side transfers on the SWDGE queue; bulk rides the two HWDGE
        # rings (sync + scalar).
        nc.gpsimd.dma_start(bqk_sb[:], bqk_ap[:])
        nc.gpsimd.dma_start(kb_sb[:], kb_ap[:])
        nc.vector.memset(warm[:], 0.0)

        # ---- phase 1: Q/K projection, 8 concurrent PSUM chains ----
        with (
            tc.tile_pool(name="pxt", bufs=1) as pxt,
            tc.tile_pool(name="pwqk", bufs=1) as pwqk,
            tc.tile_pool(name="psqk", bufs=1, space="PSUM") as psqk,
        ):
            xt_sb = pxt.tile([128, KT * S], F16, tag="xt", name="xt_sb")
            wqk_sb = pwqk.tile([128, HL * 4096], F16, tag="wqk", name="wqk_sb")

            # sync ring: everything pair-0 chains need, in kt order.
            nc.sync.dma_start(wqk_sb[:, 0:1024], wqk_ap[:, 0:1024])
            for k0, k1 in [(0, 2), (2, 5), (5, 8)]:
                nc.sync.dma_start(xt_sb[:, k0 * S:k1 * S],
                                  xt_ap[:, k0 * S:k1 * S])
            nc.sync.dma_start(wqk_sb[:, 1024:4096], wqk_ap[:, 1024:4096])
            for k0, k1 in [(8, 11), (11, 13)]:
                nc.sync.dma_start(xt_sb[:, k0 * S:k1 * S],
                                  xt_ap[:, k0 * S:k1 * S])
            # scalar ring: tail of x, remaining weight pairs, wp, wv.
            nc.scalar.dma_start(xt_sb[:, 13 * S:16 * S],
                                xt_ap[:, 13 * S:16 * S])
            for p in range(1, HL):
                nc.scalar.dma_start(wqk_sb[:, p * 4096:(p + 1) * 4096],
                                    wqk_ap[:, p * 4096:(p + 1) * 4096])
            nc.scalar.dma_start(wp_sb[:], wp_ap[:])
            nc.scalar.dma_start(wv_sb[:], wv_ap[:])

            # PE warm-up: HAM needs ~3.4us of sustained activity to lift the
            # clock gate; burn the initial DMA wait on dummy matmuls. Bank of
            # tag psqk7 is the last one the first real pair touches.
            wps = psqk.tile([128, 512], F32, tag="psqk7", bufs=1, name="wps")
            for _ in range(16):
                nc.tensor.matmul(wps[:], warm[:, 0:128], warm[:],
                                 start=True, stop=True, skip_group_check=True)

            for h in range(HL):
                ps = [psqk.tile([128, 512], F32, tag=f"psqk{i}", bufs=1,
                                name=f"psqk{i}") for i in range(8)]
                for kt in range(KT):
                    for half in range(2):
                        base = h * 4096 + kt * 256 + half * 128
                        wsl = wqk_sb[:, base:base + 128]
                        for tb in range(4):
                            nc.tensor.matmul(
                                ps[half * 4 + tb][:],
                                wsl,
                                xt_sb[:, kt * S + tb * 512:kt * S + (tb + 1) * 512],
                                start=(kt == 0), stop=(kt == KT - 1),
                                skip_group_check=True,
                            )
                # evac; for the last pair run in reverse so the high banks
                # (reused by psv) free first.
                order = list(range(8)) if h < HL - 1 else list(range(7, -1, -1))
                for i in order:
                    half, tb = divmod(i, 4)
                    dest = (qt if half == 0 else ktt)[h]
                    col = h if half == 0 else 4 + h
                    nc.scalar.add(dest[:, tb * 512:(tb + 1) * 512],
                                  ps[i][:], bqk_sb[:, col:col + 1])

        # ---- phase 2: V, attention, c_proj (pair-granularity weave) ----
        with (
            tc.tile_pool(name="p2", bufs=1) as p2,
            tc.tile_pool(name="ps2a", bufs=1, space="PSUM") as ps2a,
        ):
            e_store = {}
            rcp_store = {}
            cnt = [0]

            def s_pairs(qb, h):
                """8 closures, one score-pair each; plus a tail closure."""
                es = []
                e_store[(qb, h)] = es

                def mk(p):
                    def go():
                        pss = ps2a.tile([128, 1024], F32, tag="pss", bufs=2,
                                        name="pss")
                        for half in range(2):
                            kt = 2 * p + half
                            nc.tensor.matmul(
                                pss[:, half * 512:(half + 1) * 512],
                                ktt[h][:, kt * 128:(kt + 1) * 128],
                                qt[h][:, qb * 512:(qb + 1) * 512],
                                start=True, stop=True,
                            )
                        e = p2.tile([128, 1024], F16, tag=f"e{p}", bufs=3,
                                    name=f"e{p}")
                        nc.scalar.activation(
                            e[:], pss[:], mybir.ActivationFunctionType.Exp,
                            scale=SCALE,
                        )
                        if use_mask:
                            for half in range(2):
                                kt = 2 * p + half
                                sl = e[:, half * 512:(half + 1) * 512]
                                nc.vector.tensor_scalar_mul(
                                    sl, sl, kb_sb[:, kt:kt + 1])
                        es.append(e)
                    return go

                def tail():
                    l1s = []
                    for i in range(4):
                        l1 = p2.tile([128, 1024], F16, tag=f"l1_{i % 2}",
                                     bufs=2, name=f"l1_{i % 2}")
                        nc.vector.tensor_add(l1[:], es[2 * i][:],
                                             es[2 * i + 1][:])
                        l1s.append(l1)
                    l2s = []
                    for j in range(2):
                        l2 = p2.tile([128, 1024], F16, tag=f"l2_{j}", bufs=2,
                                     name=f"l2_{j}")
                        nc.vector.tensor_add(l2[:], l1s[2 * j][:],
                                             l1s[2 * j + 1][:])
                        l2s.append(l2)
                    l3 = p2.tile([128, 1024], F16, tag="l3", bufs=2, name="l3")
                    nc.vector.tensor_add(l3[:], l2s[0][:], l2s[1][:])
                    dn = p2.tile([128, 512], F32, tag="dn", bufs=2, name="dn")
                    nc.vector.tensor_add(dn[:], l3[:, 0:512], l3[:, 512:1024])
                    dnr = p2.tile([128, 512], F32, tag="dnr", bufs=2,
                                  name="dnr")
                    nc.gpsimd.partition_all_reduce(
                        dnr[:], dn[:], channels=128,
                        reduce_op=bass_isa.ReduceOp.add)
                    rcp = p2.tile([128, 512], F32, tag="rcp", bufs=2,
                                  name="rcp")
                    nc.vector.reciprocal_approx_fast(rcp[:], dnr[:])
                    rcp_store[(qb, h)] = rcp

                return [mk(p) for p in range(8)], tail

            def a_chunks(qb, h):
                """4 closures of 4 AV matmuls each; last one normalizes."""
                psot_box = [None]

                def mk(c):
                    def go():
                        if c == 0:
                            psot_box[0] = ps2a.tile([128, 512], F32,
                                                    tag="psot", bufs=2,
                                                    name="psot")
                        es = e_store[(qb, h)]
                        for kt in range(4 * c, 4 * c + 4):
                            nc.tensor.matmul(
                                psot_box[0][:],
                                vaug[kt][h][:],
                                es[kt // 2][:, (kt % 2) * 512:(kt % 2 + 1) * 512],
                                start=(kt == 0), stop=(kt == KT - 1),
                            )
                        if c == 3:
                            nc.vector.tensor_mul(
                                ot[h][:, qb * 512:(qb + 1) * 512],
                                psot_box[0][:], rcp_store.pop((qb, h))[:])
                            e_store.pop((qb, h))
                    return go

                return [mk(c) for c in range(4)]

            def weave(pairs_tail, units):
                pairs, tail = pairs_tail
                for i in range(8):
                    pairs[i]()
                    if i < len(units):
                        units[i]()
                for u in units[8:]:
                    u()
                tail()

            with tc.tile_pool(name="ps2b", bufs=1, space="PSUM") as ps2b:

                def v_chunks(t):
                    psv_box = [None]

                    def mk(c):
                        def go():
                            if c == 0:
                                xvt = pers.tile([128, S], F16, tag="xv",
                                                bufs=2, name="xv")
                                psv_box[0] = (
                                    ps2b.tile([128, FL], F32, tag="psv",
                                              bufs=2, name="psv"), xvt)
                                nc.sync.dma_start(xvt[:],
                                                  xv_ap[:, t * S:(t + 1) * S])
                            psv, xvt = psv_box[0]
                            for kt in range(4 * c, 4 * c + 4):
                                nc.tensor.matmul(
                                    psv[:],
                                    xvt[:, kt * 128:(kt + 1) * 128],
                                    wv_sb[:, kt * FL:(kt + 1) * FL],
                                    start=(kt == 0), stop=(kt == KT - 1),
                                )
                            if c == 3:
                                for h in range(HL):
                                    nc.vector.tensor_copy(
                                        vaug[t][h][:],
                                        psv[:, h * 128:(h + 1) * 128])
                        return go

                    return [mk(c) for c in range(4)]

                for t in range(10):
                    for u in v_chunks(t):
                        u()
                weave(s_pairs(0, 0), v_chunks(10) + v_chunks(11))
                weave(s_pairs(0, 1), v_chunks(12) + v_chunks(13))
                weave(s_pairs(0, 2), v_chunks(14) + v_chunks(15))

            weave(s_pairs(0, 3), a_chunks(0, 0) + a_chunks(0, 1))
            for u in a_chunks(0, 2):
                u()

            with tc.tile_pool(name="ps2c", bufs=1, space="PSUM") as ps2c:
                stage_box = [None]

                def c_units(t):
                    def mk(nb):
                        def go():
                            if nb == 0:
                                stage_box[0] = p2.tile([128, S], F16,
                                                       tag="stage", bufs=3,
                                                       name="stage")
                            psp = ps2c.tile([128, 512], F32,
                                            tag=f"psp{cnt[0] % 2}", bufs=1,
                                            name=f"psp{cnt[0] % 2}")
                            cnt[0] += 1
                            for h in range(HL):
                                nc.tensor.matmul(
                                    psp[:],
                                    ot[h][:, t * 128:(t + 1) * 128],
                                    wp_sb[:, h * D + nb * 512:h * D + (nb + 1) * 512],
                                    start=(h == 0), stop=(h == HL - 1),
                                    skip_group_check=True,
                                )
                            st = stage_box[0]
                            nc.vector.tensor_copy(
                                st[:, nb * 512:(nb + 1) * 512], psp[:])
                            if nb == 3:
                                eng = nc.sync if t % 2 else nc.scalar
                                eng.dma_start(
                                    out_ap[t * 128:(t + 1) * 128, :], st[:])
                        return go

                    return [mk(nb) for nb in range(4)]

                for qb in range(1, QB):
                    for h in range(HL):
                        prev = (qb - 1, 3) if h == 0 else (qb, h - 1)
                        weave(s_pairs(qb, h),
                              a_chunks(*prev) + c_units((qb - 1) * 4 + h))
                for u in a_chunks(QB - 1, 3):
                    u()
                for t in range(12, 16):
                    for u in c_units(t):
                        u()

    nc.compile()
    nc.m = get_hw_module(nc.m)
    return nc


def kernel(hidden_states, attention_mask, w_attn, b_attn, w_proj, b_proj):
    global LAST_RESULTS
    hidden_states = np.asarray(hidden_states, dtype=np.float32)
    attention_mask = np.asarray(attention_mask, dtype=np.float32)
    w_attn = np.asarray(w_attn, dtype=np.float32)
    b_attn = np.asarray(b_attn, dtype=np.float32)
    w_proj = np.asarray(w_proj, dtype=np.float32)
    b_proj = np.asarray(b_proj, dtype=np.float32)

    use_mask = bool((attention_mask != 1.0).any())
    key = ("prog", use_mask)
    if key not in _PROG_CACHE:
        _PROG_CACHE[key] = _build_program(use_mask)
    nc = _PROG_CACHE[key]

    in_maps = []
    for c in range(N_CORES):
        b, g = divmod(c, 4)
        X = np.ascontiguousarray(hidden_states[b].T).astype(NPF16)  # [D, S]
        xt = np.ascontiguousarray(
            X.reshape(KT, 128, S).transpose(1, 0, 2).reshape(128, KT * S))
        xv = np.ascontiguousarray(
            X.reshape(KT, 128, TT, 128).transpose(1, 2, 0, 3)
            .reshape(128, TT * S))
        wq = w_attn[:, g * FL:(g + 1) * FL]
        wk = w_attn[:, D + g * FL:D + (g + 1) * FL]
        wvl = w_attn[:, 2 * D + g * FL:2 * D + (g + 1) * FL]
        A = wq.reshape(KT, 128, HL, 128)
        Bm = wk.reshape(KT, 128, HL, 128)
        wqk = np.ascontiguousarray(
            np.stack([A, Bm], axis=3).transpose(1, 2, 0, 3, 4)
            .reshape(128, HL * 4096)).astype(NPF16)
        wv = np.ascontiguousarray(
            wvl.reshape(KT, 128, FL).transpose(1, 0, 2)
            .reshape(128, KT * FL)).astype(NPF16)
        wp = np.ascontiguousarray(
            w_proj[g * FL:(g + 1) * FL, :].reshape(HL, 128, D)
            .transpose(1, 0, 2).reshape(128, HL * D)).astype(NPF16)
        bq = b_attn[g * FL:(g + 1) * FL]
        bk = b_attn[D + g * FL:D + (g + 1) * FL]
        bqk = np.ascontiguousarray(
            np.concatenate([bq, bk]).reshape(8, 128).T).astype(np.float32)
        kb = np.ascontiguousarray(
            attention_mask[b].reshape(KT, 128).T).astype(np.float32)
        in_maps.append({
            "xt": xt,
            "xv": xv,
            "wqk": wqk,
            "wv": wv,
            "wp": wp,
            "bqk": bqk,
            "kb": kb,
        })

    if not os.environ.get("KERNEL_ALLOW_TRACE"):
        os.environ["BASS_NEVER_TRACE"] = "1"
    try:
        res = bass_utils.run_bass_kernel_spmd(nc, in_maps,
                                              list(range(N_CORES)))
    except Exception:
        # Transient NRT failures can leave the axon device wedged; reset it
        # once and retry. If the reset path is unavailable, the retry's own
        # failure propagates.
        try:
            import ctypes

            import jax

            jax.devices()
            _lib = ctypes.CDLL("/opt/axon/libaxon_pjrt.so")
            _lib.axon_reset.restype = ctypes.c_int64
            _lib.axon_reset()
        except Exception:
            pass
        res = bass_utils.run_bass_kernel_spmd(nc, in_maps,
                                              list(range(N_CORES)))
    LAST_RESULTS = res

    # host reduce: sum the 4 head-group partials per batch, add biases.
    # V-bias contribution: rows of A sum to 1, so each core's O gains b_v
    # per row; through c_proj that's a constant row b_v @ w_proj_slice.
    out = np.zeros((B, S, D), dtype=np.float32)
    for c in range(N_CORES):
        b, g = divmod(c, 4)
        out[b] += res.results[c]["out"].astype(np.float32)
    bias_row = b_proj.astype(np.float64).copy()
    for g in range(4):
        bv = b_attn[2 * D + g * FL:2 * D + (g + 1) * FL].astype(np.float64)
        bias_row += bv @ w_proj[g * FL:(g + 1) * FL, :].astype(np.float64)
    out += bias_row.astype(np.float32)[None, None, :]
    return out
